# revision 1
# baseline (speedup 1.0000x reference)
# Causal self-attention on 8 NeuronCores (Trainium2, Bass/Tile) — v2.
#
# Sharding: core c -> batch b = c//4, head-group hg = c%4 (4 of 16 heads).
# Each core: QKV projections for its heads, causal attention, partial
# out-projection (its heads' rows of W_out). Host sums the 4 partials per
# batch and adds b_out.
#
# vs the original baseline: the PV matmul streams O[q, D] (N=65 incl the
# softmax-denominator ones-column) instead of O^T[D, q] (N=512), cutting PE
# PSUM-row streaming ~15%; O^T for the out-projection comes from a PE-array
# transpose + ACT copy; exp is fused across each head pair (one ACT instr per
# key-tile); the causal mask is applied by Pool affine_select directly on the
# exp output; below-diagonal zero-memsets are dropped (PV never reads those
# columns); the out-projection is emitted one q-slab late so its PSUM reads
# never stall the PE; b_out is added on the host with the partial-sum combine.

import numpy as np
import ml_dtypes

import concourse.bass as bass
import concourse.mybir as mybir
import concourse.tile as tile
from concourse import bacc
from concourse.bass_utils import run_bass_kernel_spmd

F32 = mybir.dt.float32
BF16 = mybir.dt.bfloat16
AF = mybir.ActivationFunctionType
OP = mybir.AluOpType

T = 2048
E = 1024
D = 64
NH = 16
H_CORE = 4          # heads per core
PAIRS = 2           # head pairs per core
EC = E // 128       # 8 e-chunks
NT4 = T // 512      # 4 t-slabs of 512
NKT = T // 128      # 16 k-tiles of 128
NQT = T // 128      # 16 q-tiles of 128

_cache = {}


def _build(reps=1):
    nc = bacc.Bacc(None, target_bir_lowering=False)
    xT = nc.declare_dram_parameter("xT", [E, T], BF16, isOutput=False)
    wq = nc.declare_dram_parameter("wq", [E, 256], BF16, isOutput=False)
    wk = nc.declare_dram_parameter("wk", [E, 256], BF16, isOutput=False)
    wv = nc.declare_dram_parameter("wv", [E, 256], BF16, isOutput=False)
    wo = nc.declare_dram_parameter("wo", [256, E], BF16, isOutput=False)
    bq = nc.declare_dram_parameter("bq", [128, 2], F32, isOutput=False)
    bk = nc.declare_dram_parameter("bk", [128, 2], F32, isOutput=False)
    bv = nc.declare_dram_parameter("bv", [1, 256], F32, isOutput=False)
    out = nc.declare_dram_parameter("out", [T, E], BF16, isOutput=True)

    xT_r = xT.rearrange("(c p) t -> p c t", p=128)
    wq_r = wq.rearrange("(c p) m -> p c m", p=128)
    wk_r = wk.rearrange("(c p) m -> p c m", p=128)
    wv_r = wv.rearrange("(c p) m -> p c m", p=128)
    wo_r = wo.rearrange("(c p) n -> p c n", p=128)

    import contextlib

    with tile.TileContext(nc) as tc:
        with (
            tc.tile_pool(name="w", bufs=1) as w,
            tc.tile_pool(name="pt", bufs=6) as ptp,
            tc.tile_pool(name="misc", bufs=4) as misc,
            tc.For_i(0, reps, 1) if reps > 1 else contextlib.nullcontext(),
        ):
            # ---- static tiles ----
            XT = w.tile([128, EC, T], BF16)
            WQ = w.tile([128, EC, 256], BF16)
            WK = w.tile([128, EC, 256], BF16)
            WV = w.tile([128, EC, 256], BF16)
            WO = w.tile([128, 2, E], BF16)
            BQ = w.tile([128, 2], F32)
            BK = w.tile([128, 2], F32)
            BV1 = w.tile([1, 256], F32)
            # critical-path-first loads, split across both HWDGE engines
            for c in range(EC):
                nc.scalar.dma_start(WQ[:, c, :], wq_r[:, c, :])
                nc.sync.dma_start(XT[:, c, bass.ts(0, 512)],
                                  xT_r[:, c, bass.ts(0, 512)])
            for c in range(EC):
                nc.scalar.dma_start(WK[:, c, :], wk_r[:, c, :])
            nc.scalar.dma_start(BQ[:], bq[:])
            nc.scalar.dma_start(BK[:], bk[:])
            nc.scalar.dma_start(BV1[:], bv[:])
            nc.scalar.dma_start(WV[:], wv_r[:])
            for ts in range(1, NT4):
                for c in range(EC):
                    nc.sync.dma_start(XT[:, c, bass.ts(ts, 512)],
                                      xT_r[:, c, bass.ts(ts, 512)])
            nc.scalar.dma_start(WO[:], wo_r[:])
            BVB = w.tile([128, 256], F32)
            nc.gpsimd.partition_broadcast(BVB[:], BV1[:])

            # identity for PE-array transposes
            IDN = w.tile([128, 128], BF16)
            nc.vector.memset(IDN[:], 0.0)
            nc.gpsimd.affine_select(
                out=IDN[:], in_=IDN[:], compare_op=OP.not_equal, fill=1.0,
                base=0, pattern=[[-1, 128]], channel_multiplier=1,
            )

            QT = w.tile([128, PAIRS, T], BF16, tag="QT")
            KT = w.tile([128, PAIRS, T], BF16)
            # V with ones column: [t-part, kt, head, 65]
            VS = w.tile([128, NKT, H_CORE, 65], BF16)
            nc.gpsimd.memset(VS[:, :, :, 64], 1.0)
            # O^T, heads pair-stacked: [d-part, pair, t] (DMA-transposed)
            OT = w.tile([128, PAIRS, T], BF16, tag="OT")

            # ---- phase 1: QKV projections (t-sliced so ts=0 starts early) ----
            with tc.tile_pool(name="psA", bufs=3, space="PSUM") as psA:
                for ts in range(NT4):
                    sl = bass.ts(ts, 512)
                    for p in range(PAIRS):
                        pq = psA.tile([128, 512], F32, tag="qk", name="pq")
                        for e in range(EC):
                            nc.tensor.matmul(
                                pq[:], WQ[:, e, bass.ts(p, 128)], XT[:, e, sl],
                                start=(e == 0), stop=(e == EC - 1))
                        pk = psA.tile([128, 512], F32, tag="qk", name="pk")
                        for e in range(EC):
                            nc.tensor.matmul(
                                pk[:], WK[:, e, bass.ts(p, 128)], XT[:, e, sl],
                                start=(e == 0), stop=(e == EC - 1))
                        nc.vector.tensor_scalar_add(QT[:, p, sl], pq[:],
                                                    BQ[:, p:p + 1])
                        nc.vector.tensor_scalar_add(KT[:, p, sl], pk[:],
                                                    BK[:, p:p + 1])
                    for tt in range(4 * ts, 4 * ts + 4):
                        pv = psA.tile([128, 256], F32, tag="pv")
                        for e in range(EC):
                            nc.tensor.matmul(
                                pv[:], XT[:, e, bass.ts(tt, 128)], WV[:, e, :],
                                start=(e == 0), stop=(e == EC - 1))
                        nc.vector.tensor_tensor(
                            VS[:, tt, :, 0:64],
                            pv[:].rearrange("p (h d) -> p h d", h=H_CORE),
                            BVB[:].rearrange("p (h d) -> p h d", h=H_CORE),
                            OP.add)

            # ---- phase 2: attention; out-proj pipelined one slab behind ----
            with (
                tc.tile_pool(name="psS", bufs=2, space="PSUM") as psS,
                tc.tile_pool(name="psO", bufs=2, space="PSUM") as psO,
                tc.tile_pool(name="ob", bufs=3) as obp,
            ):
                def emit_outproj(qs):
                    for tt in range(4 * qs, 4 * qs + 4):
                        pu = psS.tile([128, 2, 512], F32, tag="ps", name="pu")
                        for ns in range(2):
                            for jc in range(2):
                                nc.tensor.matmul(
                                    pu[:, ns, :], OT[:, jc, bass.ts(tt, 128)],
                                    WO[:, jc, bass.ts(ns, 512)],
                                    start=(jc == 0), stop=(jc == 1))
                        ob = obp.tile([128, 1024], BF16, tag="ob", name="ob")
                        nc.vector.tensor_copy(
                            ob[:], pu[:].rearrange("p a n -> p (a n)"))
                        nc.sync.dma_start(out[bass.ts(tt, 128), :], ob[:])

                for qs in range(NT4):
                    ktmax = 4 * qs + 4
                    for p in range(PAIRS):
                        # pass A: scores (S^T layout, both heads) + exp
                        PTs = {}
                        for kt in range(ktmax):
                            d = kt - 4 * qs
                            lo = 128 * d if d > 0 else 0
                            ps2 = psS.tile([128, 2, 512], F32, tag="ps",
                                           name="ps2")
                            for h in range(2):
                                nc.tensor.matmul(
                                    ps2[:, h, lo:512],
                                    KT[bass.ts(h, 64), p, bass.ts(kt, 128)],
                                    QT[bass.ts(h, 64), p,
                                       bass.ds(qs * 512 + lo, 512 - lo)],
                                    start=True, stop=True)
                            PT2 = ptp.tile([128, 2, 512], BF16, tag="PT",
                                           name="PT2", bufs=20)
                            nc.scalar.activation(
                                PT2[:, :, lo:512], ps2[:, :, lo:512], AF.Exp)
                            if d >= 0:
                                # causal mask: zero keys (partition p) > query j
                                nc.gpsimd.affine_select(
                                    out=PT2[:, :, lo:lo + 128],
                                    in_=PT2[:, :, lo:lo + 128],
                                    compare_op=OP.is_ge, fill=0.0, base=0,
                                    pattern=[[0, 2], [1, 128]],
                                    channel_multiplier=-1)
                            PTs[kt] = PT2

                        # pass B: PV accumulation into O[q, 65] per q-tile
                        OPns = {}
                        for qt in range(4 * qs, 4 * qs + 4):
                            qoff = (qt - 4 * qs) * 128
                            pO2 = psO.tile([128, 2, 65], F32, tag="pO",
                                           name="pO2")
                            for h in range(2):
                                for kt in range(qt + 1):
                                    nc.tensor.matmul(
                                        pO2[:, h, :],
                                        PTs[kt][:, h, qoff:qoff + 128],
                                        VS[:, kt, 2 * p + h, :],
                                        start=(kt == 0), stop=(kt == qt))
                            RC = misc.tile([128, 2], F32, tag="RC", name="RC")
                            nc.vector.reciprocal(RC[:], pO2[:, :, 64])
                            OPn = misc.tile([128, 128], BF16, tag="OP",
                                            name="OPn")
                            for h in range(2):
                                nc.vector.tensor_scalar_mul(
                                    OPn[:, bass.ts(h, 64)], pO2[:, h, 0:64],
                                    RC[:, h:h + 1])
                            OPns[qt] = OPn
                        # O^T via PE-array transposes (emitted after all PV
                        # chains so they don't stall the PE on normalize),
                        # PSUM->SBUF copies on ACT
                        for qt in range(4 * qs, 4 * qs + 4):
                            OTps = psO.tile([128, 128], BF16, tag="tp",
                                            name="OTps")
                            nc.tensor.transpose(OTps[:], OPns[qt][:], IDN[:])
                            nc.scalar.copy(
                                OT[:, p, bass.ds(qt * 128, 128)], OTps[:])

                    if qs > 0:
                        emit_outproj(qs - 1)
                emit_outproj(NT4 - 1)

    nc.compile()
    return nc


def _in_maps(x, W_qkv, b_qkv, W_out, b_out):
    bf = ml_dtypes.bfloat16
    scale = np.float32(1.0 / np.sqrt(D))
    maps = []
    for c in range(8):
        b, hg = c // 4, c % 4
        qc = slice(hg * 256, hg * 256 + 256)
        m = {
            "xT": np.ascontiguousarray(x[b].T).astype(bf),
            "wq": (W_qkv[:, qc.start:qc.stop] * scale).astype(bf),
            "wk": W_qkv[:, E + qc.start:E + qc.stop].astype(bf),
            "wv": W_qkv[:, 2 * E + qc.start:2 * E + qc.stop].astype(bf),
            "wo": np.ascontiguousarray(W_out[qc, :]).astype(bf),
            "bq": (b_qkv[qc] * scale).astype(np.float32).reshape(2, 128).T.copy(),
            "bk": b_qkv[E + qc.start:E + qc.stop].astype(np.float32).reshape(2, 128).T.copy(),
            "bv": b_qkv[2 * E + qc.start:2 * E + qc.stop].astype(np.float32).reshape(1, 256).copy(),
        }
        maps.append(m)
    return maps


def kernel(x, W_qkv, b_qkv, W_out, b_out):
    x = np.asarray(x, np.float32)
    W_qkv = np.asarray(W_qkv, np.float32)
    b_qkv = np.asarray(b_qkv, np.float32)
    W_out = np.asarray(W_out, np.float32)
    b_out = np.asarray(b_out, np.float32)
    if "nc" not in _cache:
        _cache["nc"] = _build()
    nc = _cache["nc"]
    maps = _in_maps(x, W_qkv, b_qkv, W_out, b_out)
    res = run_bass_kernel_spmd(nc, maps, list(range(8))).results
    out = np.empty((2, T, E), np.float32)
    for b in range(2):
        acc = res[b * 4]["out"].astype(np.float32)
        for hg in range(1, 4):
            acc = acc + res[b * 4 + hg]["out"].astype(np.float32)
        out[b] = acc + b_out.astype(np.float32)
    return out

